# revision 63
# baseline (speedup 1.0000x reference)
"""Trainium2 Bass kernel for nn_Attention (sparse_attention, 8 NeuronCores).

Sharding: data-parallel over batch (4) x tensor-parallel over heads (2 groups
of 4 heads) = 8 cores. Each core computes attention for one batch and 4 heads
entirely in transposed (feature-major) layout, so no on-chip transposes are
needed. exp(attn_bias) is precomputed on the host in bf16, so on-chip softmax
is exp(S) * expB with no PSUM-blocking adds. Wo is row-sharded; each core
returns one bf16 partial per head-pair and the host reduces.

v4: head-major unit stream (h0..h3 x 8 key-tiles). All projections run
up-front in a tight PE pipeline so the attention-phase psA ring holds ONLY
the S tiles (the PE is in-order, so any interleaved PSUM alloc breaks the
double-buffer alternation and serializes QK behind exp). The attention
phase is ACT(exp)-paced; epilogues read U straight from PSUM, the 1/den
broadcast and g*rec product run on the idle GPSIMD engine, and pair-0's
output projection borrows uv-ring PSUM slots in FIFO-consistent order.
Bias tiles are per (head, jt) on the SP ring (Pool's SWDGE costs ~1us of
engine time per DMA, so only out-DMAs ride it).
"""

import os
import sys

for _p in ("/opt/trn_rl_repo", "/root/.axon_site/_ro/trn_rl_repo"):
    if os.path.isdir(_p) and _p not in sys.path:
        sys.path.append(_p)

import numpy as np

B, N, DIM, H, DH = 4, 1024, 512, 8, 64
SCALE = DH**-0.5
HL = 4  # heads per core
HDL = HL * DH  # 256 head-dims per core
NCORES = 8
NJT = N // 128  # 8 key-tiles
NKT = DIM // 128  # 4 contraction tiles

# wpack (bf16) column layout, ordered by first use
_WQK0 = 0  # wq_p0 4kt x 128 | wk_p0 4kt x 128
_XT = 1024  # 4 kt x 1024 tokens
_WV = 5120  # 4 kt x 256
_WQK1 = 6144  # wq_p1 | wk_p1
_WG = 7168  # 4 kt x 256
_WO = 8192  # 2 pair x 512
WPC = 9216

_CACHE = {}


def _build(loop_iters=1):
    import concourse.tile as tile
    from concourse import bacc, mybir

    fp32 = mybir.dt.float32
    f32r = mybir.dt.float32r
    bf16 = mybir.dt.bfloat16

    Exp = mybir.ActivationFunctionType.Exp
    Identity = mybir.ActivationFunctionType.Identity
    mult = mybir.AluOpType.mult

    nc = bacc.Bacc("TRN2", target_bir_lowering=False, debug=False, num_devices=NCORES)

    wpack = nc.dram_tensor("wpack", [128, WPC], bf16, kind="ExternalInput").ap()
    wsmall = nc.dram_tensor("wsmall", [128, 66], f32r, kind="ExternalInput").ap()
    # bias: exp(attn_bias) transposed, per (head, jt): [4, 8, 128, N]
    expB = nc.dram_tensor("expB", [HL, NJT // 2, 128, 2 * N], bf16, kind="ExternalInput").ap()
    outT = nc.dram_tensor("outT", [2, 4, 128, N], bf16, kind="ExternalOutput").ap()

    from contextlib import ExitStack

    # unroll the loop body 2x so double-buffered tiles rotate across
    # iterations (a hardware loop reuses static SBUF addresses, so a single
    # body would serialize on its weight tiles)
    unroll = 2 if (loop_iters > 1 and loop_iters % 2 == 0) else 1

    with tile.TileContext(nc) as tc, ExitStack() as stack:
        if loop_iters > 1:
            stack.enter_context(
                tc.For_i(0, loop_iters // unroll, 1, hint_engines=(mybir.EngineType.PE, mybir.EngineType.Activation, mybir.EngineType.DVE, mybir.EngineType.SP, mybir.EngineType.Pool))
            )
        with (
            tc.tile_pool(name="const", bufs=2) as cpool,
            tc.tile_pool(name="proj", bufs=2) as projpool,
            tc.tile_pool(name="bias", bufs=8) as biaspool,
            tc.tile_pool(name="etile", bufs=4) as epool,
            tc.tile_pool(name="work", bufs=2) as workpool,
            tc.tile_pool(name="psA", bufs=2, space="PSUM") as psA,
            tc.tile_pool(name="psB", bufs=2, space="PSUM") as psB,
        ):
            def body(first=True):
                # ---- SBUF homes for weights ----
                wp_sb = cpool.tile([128, WPC], bf16, tag="wp")
                ws_sb = cpool.tile([128, 66], f32r, tag="ws")
                bg_sb = ws_sb[:, 0:2]
                ones_sb = ws_sb[:, 2:66]

                def wq(p, kt):  # [128, 128] stationary for q proj of pair p
                    base = (_WQK0 if p == 0 else _WQK1) + kt * 128
                    return wp_sb[:, base : base + 128]

                def wk(p, kt):
                    base = (_WQK0 if p == 0 else _WQK1) + 512 + kt * 128
                    return wp_sb[:, base : base + 128]

                def xT(kt, lo, size):
                    # ih-major x layout: [ih, kt, 512] so the x stream is two
                    # 512KB DMAs (HWDGE descriptor-gen is 625ns per DMA)
                    ih, l2 = divmod(lo, 512)
                    assert l2 + size <= 512
                    base = _XT + ih * 2048 + kt * 512 + l2
                    return wp_sb[:, base : base + size]

                def wv(kt):
                    return wp_sb[:, _WV + kt * 256 : _WV + (kt + 1) * 256]

                def wg_(kt, mt):
                    base = _WG + kt * 256 + mt * 128
                    return wp_sb[:, base : base + 128]

                def wo_(p, mt):
                    base = _WO + p * 512 + mt * 128
                    return wp_sb[:, base : base + 128]

                # ---- DMA issue order (SP ring => priority order) ----
                nc.sync.dma_start(ws_sb[:], wsmall)
                # PE p-state prewarm: dummy matmuls off a memset tile keep
                # the PE busy through the DMA head so the projections run at
                # full clock; the Exp LUT preload rides the same window.
                if first:
                    wtile = cpool.tile([1, 512], bf16, tag="wtile")
                    nc.gpsimd.memset(wtile[:], 0.5)  # Pool: ready ~0.7us sooner
                    for _ in range(11):
                        pw = psA.tile([1, 512], fp32, tag="big", name="pw")
                        nc.tensor.matmul(pw[:], wtile[0:1, 0:1], wtile[:], start=True, stop=True)
                lutw = cpool.tile([1, 2], fp32, tag="lut")
                nc.scalar.activation(lutw[0:1, 0:1], ones_sb[0:1, 0:1], Exp)
                # lhsT/rhs for the tail keep-warm matmuls
                wtl = cpool.tile([1, 512], bf16, tag="warmtl")
                nc.vector.memset(wtl[:], 0.25)
                # fp32 copy of bg (tensor_scalar_add requires an fp32 scalar)
                bgf = cpool.tile([128, 2], fp32, tag="bgf")
                nc.vector.tensor_copy(bgf[:], bg_sb)

                def wdma(lo, hi):
                    nc.sync.dma_start(wp_sb[:, lo:hi], wpack[:, lo:hi])

                bias_tiles = {}
                bias_full = {}

                def bdma(h, k):
                    bt = biaspool.tile([128, 2 * N], bf16, tag="bias", name=f"bt{h}_{k}")
                    nc.sync.dma_start(bt[:], expB[h, k])
                    bias_tiles[(h, 2 * k)] = bt[:, 0:N]
                    bias_tiles[(h, 2 * k + 1)] = bt[:, N : 2 * N]
                    bias_full[(h, k)] = bt

                wdma(_WQK0, _XT)  # wq_p0 | wk_p0 (512KB)
                for c in range(4):  # x in 256KB chunks, ih-major
                    wdma(_XT + c * 1024, _XT + (c + 1) * 1024)
                wdma(_WV, _WG)  # wv | wq_p1 | wk_p1 (512KB)
                wdma(_WG, WPC)  # wg | wo (512KB)
                for h in range(HL):
                    for k in range(NJT // 2):
                        bdma(h, k)

                # ---- projection targets ----
                qT_sb = [projpool.tile([128, N], bf16, tag=f"qT{m}", name=f"qT{m}") for m in range(2)]
                kT_sb = [projpool.tile([128, N], bf16, tag=f"kT{m}", name=f"kT{m}") for m in range(2)]
                gT_sb = [projpool.tile([128, N], fp32, tag=f"gT{m}", name=f"gT{m}") for m in range(2)]

                def projqk(which, p, evac_eng="vector", split=None):
                    """q (which=0) or k (which=1) projection for pair p."""
                    wsel = wq if which == 0 else wk
                    dst = (qT_sb if which == 0 else kT_sb)[p]
                    ps = psA.tile([128, N], fp32, tag="big", name="ps")
                    for ih in range(2):  # ih-major: chase the x DMA stream
                        for kt in range(NKT):
                            nc.tensor.matmul(
                                ps[:, ih * 512 : ih * 512 + 512],
                                wsel(p, kt),
                                xT(kt, ih * 512, 512),
                                start=(kt == 0),
                                stop=(kt == NKT - 1),
                            )
                    if evac_eng == "mixed":
                        # halves on ACT+DVE in parallel: these evacs gate the
                        # first S tiles via the psA ring
                        nc.scalar.copy(dst[:, 0:512], ps[:, 0:512])
                        nc.vector.tensor_copy(dst[:, 512:N], ps[:, 512:N])
                        return
                    cp = nc.scalar.copy if evac_eng == "scalar" else nc.vector.tensor_copy
                    for lo, hi in split or [(0, N)]:
                        cp(dst[:, lo:hi], ps[:, lo:hi])

                def gproj(mt):
                    ps = psA.tile([128, N], fp32, tag="big", name="psg")
                    for ih in range(2):
                        for kt in range(NKT):
                            nc.tensor.matmul(
                                ps[:, ih * 512 : ih * 512 + 512],
                                wg_(kt, mt),
                                xT(kt, ih * 512, 512),
                                start=(kt == 0),
                                stop=(kt == NKT - 1),
                            )
                    # DVE, not ACT: an ACT evac here would queue in front of
                    # the first exps and delay the attention stream start
                    nc.vector.tensor_scalar_add(
                        gT_sb[mt][:], ps[:], bgf[:, mt : mt + 1]
                    )

                # ---- v natural [token, d] + ones column per head (bf16) ----
                vhat_all = projpool.tile([128, NJT * HL * 65], bf16, tag="vhat")
                ones_view = vhat_all[:].rearrange(
                    "p (j h c) -> p j h c", j=NJT, c=65
                )[:, :, :, 64:65]
                nc.vector.memset(ones_view, 1.0)

                def vproj(jt):
                    vv = vhat_all[:, jt * HL * 65 : (jt + 1) * HL * 65].rearrange(
                        "p (h c) -> p h c", h=HL
                    )
                    ps2 = psA.tile([128, HDL], fp32, tag="big", name="ps2")
                    for kt in range(NKT):
                        nc.tensor.matmul(
                            ps2[:],
                            xT(kt, jt * 128, 128),
                            wv(kt),
                            start=(kt == 0),
                            stop=(kt == NKT - 1),
                        )
                    nc.vector.tensor_copy(
                        vv[:, :, 0:64], ps2[:].rearrange("p (h c) -> p h c", h=HL)
                    )

                # ---- projection phase: everything up-front, PE-paced.
                # g before q1/k1: the last two projections' evacs gate the
                # first two S tiles via the psA ring, and q/k evacs are fast.
                projqk(0, 0, "vector", split=[(0, 512), (512, N)])
                projqk(1, 0, "vector", split=[(0, 128), (128, N)])
                for jt in range(NJT):
                    vproj(jt)
                gproj(0)
                gproj(1)
                projqk(0, 1, "mixed")
                projqk(1, 1, "mixed")

                # ---- shared state ----
                ug_sb = [
                    workpool.tile([128, N], bf16, tag=f"ug{p}", name=f"ug{p}", bufs=2)
                    for p in range(2)
                ]
                state = {}
                uv_tiles = {}

                def u_evac(h):
                    """Copy uv[h] (U + den row) to SBUF (DVE -- GPSIMD cannot
                    access PSUM), freeing the psB slot for the next uv
                    alloc."""
                    U = workpool.tile([65, N], fp32, tag="U", name=f"U{h}", bufs=2)
                    nc.vector.tensor_copy(U[:], uv_tiles[h][0:65, :])
                    state[("U", h)] = U

                def epi_steps(h, from_sbuf=False):
                    """Per-head epilogue. rec (DVE) -> broadcast (GPSIMD for
                    h0-h2; PE matmul into the free psA ring for h3's
                    latency-critical tail) -> gs -> ug (last reader of uv)."""
                    p, hh = divmod(h, 2)

                    def src():
                        return state[("U", h)] if from_sbuf else uv_tiles[h]

                    def s_rec():
                        rec = workpool.tile([1, N], f32r, tag="rec", name="rec", bufs=4)
                        with nc.allow_low_precision(reason="feeds broadcast"):
                            nc.vector.reciprocal(rec[:], src()[64:65, :])
                        state[("rec", h)] = rec

                    def s_bc():
                        # PE broadcast of 1/den into a psA ring slot
                        # (GPSIMD is too slow on real HW and cannot access
                        # PSUM). Costs one ring insertion mid-attention.
                        bc = psA.tile([64, N], fp32, tag="big", name=f"bc{h}")
                        rec = state[("rec", h)]
                        for ih in range(2):
                            nc.tensor.matmul(
                                bc[:, ih * 512 : ih * 512 + 512],
                                ones_sb[0:1, 0:64],
                                rec[0:1, ih * 512 : ih * 512 + 512],
                                start=True,
                                stop=True,
                            )
                        state[("gbc", h)] = bc

                    def s_gs():
                        gs = workpool.tile([64, N], fp32, tag="gs", name="gs", bufs=4)
                        # in0 is PSUM (PE broadcast) -> exempt from the SBUF
                        # base-partition rule
                        nc.vector.tensor_tensor(
                            out=gs[:],
                            in0=state[("gbc", h)][:],
                            in1=gT_sb[p][hh * 64 : hh * 64 + 64, :],
                            op=mult,
                        )
                        state[("gs", h)] = gs

                    def s_ug():
                        # U straight from PSUM (h2: from its SBUF copy, made
                        # to free the psB slot for h3's uv)
                        nc.vector.tensor_tensor(
                            out=ug_sb[p][hh * 64 : hh * 64 + 64, :],
                            in0=src()[0:64, :],
                            in1=state[("gs", h)][:],
                            op=mult,
                        )

                    return [s_rec, s_bc, s_gs, s_ug]

                op_state = {}

                def op0_alloc(key="t"):
                    op_state[key] = psB.tile([128, N], fp32, tag="uv", name="opt")

                def outproj_step(p, mt, ps=None, evac="vector", ring="gpsimd"):
                    ps = op_state["t2" if mt >= 2 else "t"] if ps is None else ps
                    lhsT = wo_(p, mt)
                    for ih in range(2):
                        nc.tensor.matmul(
                            ps[:, ih * 512 : ih * 512 + 512],
                            lhsT,
                            ug_sb[p][:, ih * 512 : ih * 512 + 512],
                            start=True,
                            stop=True,
                        )
                    ot = workpool.tile([128, N], bf16, tag="osb", name="osb", bufs=4)
                    if evac == "halves":
                        # ACT+DVE in parallel: halves the tail evac latency
                        nc.scalar.copy(ot[:, 0:512], ps[:, 0:512])
                        nc.vector.tensor_copy(ot[:, 512:N], ps[:, 512:N])
                    elif evac == "scalar":
                        nc.scalar.copy(ot[:], ps[:])
                    elif evac == "gpsimd":
                        nc.gpsimd.tensor_copy(ot[:], ps[:])
                    else:
                        nc.vector.tensor_copy(ot[:], ps[:])
                    (nc.gpsimd if ring == "gpsimd" else nc.sync).dma_start(
                        outT[p, mt], ot[:]
                    )

                # ---- per-unit background thunks (no PSUM from psA!) ----
                e0, e1_, e2, e3 = (epi_steps(h, from_sbuf=(h == 2)) for h in range(4))
                # NOTE: head h's last AV flush happens inside unit 8*h+10
                # (lag 3), BEFORE that unit's thunks run -- epilogue reads of
                # uv[h] must be issued at or after that unit, else the read
                # lands mid-accumulation-group.
                sched = {
                    # head h's last AV flush lands in unit 8*h+11 with lag=4
                    12: [e0[0]],
                    13: [e0[1]],
                    14: [e0[2]],
                    15: [e0[3]],
                    19: [e1_[0]],
                    20: [e1_[1]],
                    21: [e1_[2]],
                    22: [e1_[3], op0_alloc],
                    24: [lambda: outproj_step(0, 0)],
                    25: [lambda: outproj_step(0, 1), lambda: op0_alloc("t2")],
                    26: [lambda: outproj_step(0, 2, ring="sync")],
                    27: [lambda: outproj_step(0, 3, ring="sync"), e2[0]],
                    # u_evac(2) is placed inline after its last AV flush (u26)
                    28: [e2[1]],
                    29: [e2[2]],
                    31: [e2[3]],
                }

                # ---- the 32-unit attention stream ----
                pend = []

                def flush_av():
                    h0_, jt0, e_t = pend.pop(0)
                    uv = uv_tiles[h0_]
                    base = jt0 * HL * 65 + h0_ * 65
                    for ih in range(2):
                        nc.tensor.matmul(
                            uv[0:65, ih * 512 : ih * 512 + 512],
                            vhat_all[:, base : base + 65],
                            e_t[:, ih * 512 : ih * 512 + 512],
                            start=(jt0 == 0),
                            stop=(jt0 == NJT - 1),
                        )
                    return (h0_, jt0)

                for h in range(HL):
                    p, hh = divmod(h, 2)
                    uv_tiles[h] = psB.tile([128, N], fp32, tag="uv", name=f"uv{h}")
                    lag = 4
                    for jt in range(NJT):
                        u = 8 * h + jt
                        st = psA.tile([128, N], fp32, tag="big", name=f"st{u}")
                        lhsT = kT_sb[p][hh * 64 : hh * 64 + 64, jt * 128 : jt * 128 + 128]
                        for ih in range(2):
                            nc.tensor.matmul(
                                st[:, ih * 512 : ih * 512 + 512],
                                lhsT,
                                qT_sb[p][hh * 64 : hh * 64 + 64, ih * 512 : ih * 512 + 512],
                                start=True,
                                stop=True,
                            )
                        if jt % 2 == 0:
                            e1t = epool.tile(
                                [128, 2 * N], bf16, tag="e1", name="e1", bufs=2
                            )
                            state[("e1", h)] = e1t
                        else:
                            e1t = state[("e1", h)]
                        nc.scalar.activation(
                            e1t[:, (jt % 2) * N : (jt % 2) * N + N], st[:], Exp
                        )
                        if jt % 2 == 1:
                            # one fused bias-multiply per jt-pair: fewer DVE
                            # instructions, same bytes
                            et = epool.tile(
                                [128, 2 * N], bf16, tag="e", name="e", bufs=3
                            )
                            btp = bias_full[(h, jt // 2)]
                            nc.vector.tensor_tensor(
                                out=et[:], in0=e1t[:], in1=btp[:], op=mult
                            )
                            pend.append((h, jt - 1, et[:, 0:N]))
                            pend.append((h, jt, et[:, N : 2 * N]))
                        while len(pend) > lag:
                            done = flush_av()
                            if done == (2, NJT - 1):
                                u_evac(2)  # free h2's psB slot for h3
                        for th in sched.get(u, []):
                            th()

                # ---- tail: drain AVs, epi h3, outproj p1 ----
                while pend:
                    done = flush_av()
                    if done == (2, NJT - 1):
                        u_evac(2)
                e3[0]()  # rec h3 (DVE)
                # outproj p1: four independent PSUM tiles (two psA slots plus
                # two fresh psB-ring slots, all free by now) -> no ring waits
                po_tiles = [
                    psA.tile([128, N], fp32, tag="big", name="po0"),
                    psA.tile([128, N], fp32, tag="big", name="po1"),
                    psB.tile([128, N], fp32, tag="uv", name="po2"),
                    psB.tile([128, N], fp32, tag="uv", name="po3"),
                ]
                # keep-warm matmuls: without them the PE idles through the
                # rec/gs chain, drops p-state, and the final outproj runs at
                # 1/2..1/4 clock. They write into po3 -- freshly allocated,
                # no pending readers to race with; p1-mt3's start=True
                # resets it before real use. Split around bc3 so the PE
                # stream is continuous from the AV drain on.
                opt = po_tiles[3]

                def warm(k):
                    for _ in range(k):
                        nc.tensor.matmul(
                            opt[0:1, 0:512], wtl[0:1, 0:1], wtl[:], start=True, stop=True
                        )

                warm(5)
                e3[1]()  # bc h3 (PE, psA ring is free now)
                e3[2]()  # gs h3 (DVE)
                warm(12)
                e3[3]()  # ug h3 (DVE)
                for mt in range(4):
                    outproj_step(
                        1,
                        mt,
                        ps=po_tiles[mt],
                        evac="halves",
                        ring="sync" if mt % 2 == 0 else "gpsimd",
                    )

            for i in range(unroll):
                body(first=(i == 0))

    nc.compile()
    return nc


def _shard_inputs(x, attn_bias, Wq, Wkv, Wg, bg, Wo):
    """Build per-core input maps (host-side layout prep)."""
    import ml_dtypes

    bf16 = ml_dtypes.bfloat16

    def kmaj(w):  # [512, F] -> [128, NKT*F] contraction-tile-major
        f = w.shape[1]
        return np.ascontiguousarray(
            w.reshape(NKT, 128, f).transpose(1, 0, 2)
        ).reshape(128, NKT * f)

    in_maps = []
    for d in range(NCORES):
        b, g = d // 2, d % 2
        cs = slice(g * HDL, (g + 1) * HDL)
        xTh = np.ascontiguousarray(x[b].T)  # [512, 1024]
        wq_all = Wq[:, cs] * SCALE  # [512, 256]
        wk_all = Wkv[:, g * HDL : (g + 1) * HDL]
        wv_all = Wkv[:, H * DH + g * HDL : H * DH + (g + 1) * HDL]
        wg_all = Wg[:, cs]
        # x in ih-major chunk order: [128, (ih, kt, tok512)]
        x_ihmaj = np.ascontiguousarray(
            xTh.reshape(NKT, 128, 2, 512).transpose(1, 2, 0, 3)
        ).reshape(128, 2 * NKT * 512)
        chunks = [
            kmaj(wq_all[:, 0:128]),
            kmaj(wk_all[:, 0:128]),
            x_ihmaj,
            kmaj(wv_all),
            kmaj(wq_all[:, 128:256]),
            kmaj(wk_all[:, 128:256]),
            kmaj(wg_all),
            np.ascontiguousarray(
                Wo[cs, :].reshape(2, 128, DIM).transpose(1, 0, 2).reshape(128, 2 * DIM)
            ),
        ]
        wpack = np.concatenate(chunks, axis=1).astype(bf16)
        wsmall = np.concatenate(
            [
                np.ascontiguousarray(bg[cs].reshape(2, 128).T),
                np.ones((128, 64), np.float32),
            ],
            axis=1,
        ).astype(np.float32)

        ab = attn_bias[b, g * HL : (g + 1) * HL]  # [4, N(i), N(j)]
        # per (head, jt-pair): [4, 4, 128(jpart), 2048] of exp(bias^T)
        abT = ab.transpose(0, 2, 1).reshape(HL, NJT // 2, 2, 128, N)
        eB = np.exp(abT.transpose(0, 1, 3, 2, 4))
        expB = np.ascontiguousarray(eB).reshape(HL, NJT // 2, 128, 2 * N).astype(bf16)
        in_maps.append({"wpack": wpack, "wsmall": wsmall, "expB": expB})
    return in_maps


def _unshard(results, bo):
    out = np.empty((B, N, DIM), dtype=np.float32)
    for b in range(B):
        acc = results[2 * b]["outT"].astype(np.float32).sum(axis=0) + results[
            2 * b + 1
        ]["outT"].astype(np.float32).sum(axis=0)
        out[b] = acc.reshape(DIM, N).T + bo[None, :]
    return out


def kernel(x, mask, attn_bias, Wq, Wkv, Wg, bg, Wo, bo):
    """Full inputs in, full output out. mask is all-ones by construction."""
    from concourse.bass_utils import run_bass_kernel_spmd

    x = np.asarray(x, dtype=np.float32)
    attn_bias = np.asarray(attn_bias, dtype=np.float32)
    Wq = np.asarray(Wq, dtype=np.float32)
    Wkv = np.asarray(Wkv, dtype=np.float32)
    Wg = np.asarray(Wg, dtype=np.float32)
    bg = np.asarray(bg, dtype=np.float32)
    Wo = np.asarray(Wo, dtype=np.float32)
    bo = np.asarray(bo, dtype=np.float32)

    if "nc" not in _CACHE:
        _CACHE["nc"] = _build()
    in_maps = _shard_inputs(x, attn_bias, Wq, Wkv, Wg, bg, Wo)
    res = run_bass_kernel_spmd(_CACHE["nc"], in_maps, core_ids=list(range(NCORES)))
    return _unshard(res.results, bo)


# revision 65
# speedup vs baseline: 1.1468x; 1.1468x over previous
"""Trainium2 Bass kernel for nn_Attention (sparse_attention, 8 NeuronCores).

Sharding: data-parallel over batch (4) x tensor-parallel over heads (2 groups
of 4 heads) = 8 cores. Each core computes attention for one batch and 4 heads
entirely in transposed (feature-major) layout, so no on-chip transposes are
needed. exp(attn_bias) is precomputed on the host in bf16, so on-chip softmax
is exp(S) * expB with no PSUM-blocking adds. Wo is row-sharded; each core
returns one bf16 partial per head-pair and the host reduces.

v3: all weights/x in bf16 (halves the weight DMA); DMA issue order tuned so
the first QK starts early and bias chunks stream per (pair, jt, head); the
exp LUT is preloaded and the PE p-state prewarmed during the DMA head; AV
matmuls run 3 units behind their QK so the in-order PE never waits on the
exp/mult chain; the last pair's epilogue interleaves into the AV drain; for
even timing loops the body is unrolled 2x with double-buffered weight and
projection tiles (and output DMAs on the gpsimd ring) so iteration n+1's
DMA head overlaps iteration n's compute tail.
"""

import os
import sys

for _p in ("/opt/trn_rl_repo", "/root/.axon_site/_ro/trn_rl_repo"):
    if os.path.isdir(_p) and _p not in sys.path:
        sys.path.append(_p)

import numpy as np

B, N, DIM, H, DH = 4, 1024, 512, 8, 64
SCALE = DH**-0.5
HL = 4  # heads per core
HDL = HL * DH  # 256 head-dims per core
NCORES = 8
NJT = N // 128  # 8 key-tiles
NKT = DIM // 128  # 4 contraction tiles

# wpack (bf16) column layout, ordered by first use
_WQK0 = 0  # wq_p0 4kt x 128 | wk_p0 4kt x 128
_XT = 1024  # 4 kt x 1024 tokens
_WV = 5120  # 4 kt x 256
_WQK1 = 6144  # wq_p1 | wk_p1
_WG = 7168  # 4 kt x 256
_WO = 8192  # 2 pair x 512
WPC = 9216

_CACHE = {}


def _build(loop_iters=1):
    import concourse.tile as tile
    from concourse import bacc, mybir

    fp32 = mybir.dt.float32
    f32r = mybir.dt.float32r
    bf16 = mybir.dt.bfloat16

    Exp = mybir.ActivationFunctionType.Exp
    Identity = mybir.ActivationFunctionType.Identity
    mult = mybir.AluOpType.mult

    nc = bacc.Bacc("TRN2", target_bir_lowering=False, debug=False, num_devices=NCORES)

    wpack = nc.dram_tensor("wpack", [128, WPC], bf16, kind="ExternalInput").ap()
    wsmall = nc.dram_tensor("wsmall", [128, 66], f32r, kind="ExternalInput").ap()
    expB = nc.dram_tensor("expB", [2, NJT, 128, 2 * N], bf16, kind="ExternalInput").ap()
    outT = nc.dram_tensor("outT", [2, 4, 128, N], bf16, kind="ExternalOutput").ap()

    from contextlib import ExitStack

    # unroll the loop body 2x so double-buffered tiles rotate across
    # iterations (a hardware loop reuses static SBUF addresses, so a single
    # body would serialize on its weight tiles)
    unroll = 2 if (loop_iters > 1 and loop_iters % 2 == 0) else 1

    with tile.TileContext(nc) as tc, ExitStack() as stack:
        if loop_iters > 1:
            stack.enter_context(
                tc.For_i(0, loop_iters // unroll, 1, hint_engines=(mybir.EngineType.PE, mybir.EngineType.Activation, mybir.EngineType.DVE, mybir.EngineType.SP, mybir.EngineType.Pool))
            )
        with (
            tc.tile_pool(name="const", bufs=2) as cpool,
            tc.tile_pool(name="proj", bufs=2) as projpool,
            tc.tile_pool(name="bias", bufs=10) as biaspool,
            tc.tile_pool(name="etile", bufs=4) as epool,
            tc.tile_pool(name="work", bufs=2) as workpool,
            tc.tile_pool(name="psA", bufs=2, space="PSUM") as psA,
            tc.tile_pool(name="psB", bufs=2, space="PSUM") as psB,
        ):
            def body(first=True):
                # ---- SBUF homes for weights ----
                wp_sb = cpool.tile([128, WPC], bf16, tag="wp")
                ws_sb = cpool.tile([128, 66], f32r, tag="ws")
                bg_sb = ws_sb[:, 0:2]
                ones_sb = ws_sb[:, 2:66]

                def wq(p, kt):  # [128, 128] stationary for q proj of pair p
                    base = (_WQK0 if p == 0 else _WQK1) + kt * 128
                    return wp_sb[:, base : base + 128]

                def wk(p, kt):
                    base = (_WQK0 if p == 0 else _WQK1) + 512 + kt * 128
                    return wp_sb[:, base : base + 128]

                def xT(kt, lo, size):
                    return wp_sb[:, _XT + kt * 1024 + lo : _XT + kt * 1024 + lo + size]

                def wv(kt):
                    return wp_sb[:, _WV + kt * 256 : _WV + (kt + 1) * 256]

                def wg_(kt, mt):
                    base = _WG + kt * 256 + mt * 128
                    return wp_sb[:, base : base + 128]

                def wo_(p, mt):
                    base = _WO + p * 512 + mt * 128
                    return wp_sb[:, base : base + 128]

                # ---- DMA issue order (single sync ring => priority) ----
                nc.sync.dma_start(ws_sb[:], wsmall)
                # PE p-state prewarm: dummy matmuls off a memset tile keep
                # the PE busy through the DMA head so the projections run at
                # full clock; the Exp LUT preload rides the same window.
                if first:
                    wtile = cpool.tile([1, 512], bf16, tag="wtile")
                    nc.vector.memset(wtile[:], 0.5)
                    for _ in range(11):
                        pw = psA.tile([1, 512], fp32, tag="big", name="pw")
                        nc.tensor.matmul(pw[:], wtile[0:1, 0:1], wtile[:], start=True, stop=True)
                lutw = cpool.tile([1, 2], fp32, tag="lut")
                nc.scalar.activation(lutw[0:1, 0:1], ones_sb[0:1, 0:1], Exp)
                # lhsT/rhs for the tail keep-warm matmuls
                wtl = cpool.tile([1, 512], bf16, tag="warmtl")
                nc.vector.memset(wtl[:], 0.25)

                def wdma(lo, hi):
                    nc.sync.dma_start(wp_sb[:, lo:hi], wpack[:, lo:hi])

                bias_tiles = {}

                def bdma(p, jt, split=False):
                    bt = biaspool.tile([128, 2 * N], bf16, tag="bias", name=f"bt{p}_{jt}")
                    ring = nc.gpsimd if p == 1 else nc.sync
                    if split:
                        for hh in range(2):
                            ring.dma_start(
                                bt[:, hh * N : (hh + 1) * N],
                                expB[p, jt, :, hh * N : (hh + 1) * N],
                            )
                    else:
                        ring.dma_start(bt[:], expB[p, jt])
                    bias_tiles[(p, jt)] = bt

                wdma(_WQK0, _XT)  # wq_p0 | wk_p0
                for ih in range(2):  # ih-major half-chunks: q0's ih0 matmuls
                    for kt in range(NKT):  # start after half the xT stream
                        lo = _XT + kt * 1024 + ih * 512
                        wdma(lo, lo + 512)
                bdma(0, 0, split=True)
                wdma(_WV, _WQK1)
                bdma(0, 1, split=True)
                wdma(_WQK1, _WG)
                wdma(_WG, _WO)
                bdma(0, 2)
                wdma(_WO, WPC)
                for jt in range(3, NJT):
                    bdma(0, jt)
                for jt in range(NJT):
                    bdma(1, jt)

                # ---- projections ----
                qT_sb = [projpool.tile([128, N], bf16, tag=f"qT{m}", name=f"qT{m}") for m in range(2)]
                kT_sb = [projpool.tile([128, N], bf16, tag=f"kT{m}", name=f"kT{m}") for m in range(2)]
                gT_sb = [projpool.tile([128, N], fp32, tag=f"gT{m}", name=f"gT{m}") for m in range(2)]

                def projqk(which, p, evac_eng, split=None):
                    """q (which=0) or k (which=1) projection for pair p."""
                    wsel = wq if which == 0 else wk
                    dst = (qT_sb if which == 0 else kT_sb)[p]
                    ps = psA.tile([128, N], fp32, tag="big", name="ps")
                    for kt in range(NKT):
                        lhsT = wsel(p, kt)
                        for ih in range(2):
                            nc.tensor.matmul(
                                ps[:, ih * 512 : ih * 512 + 512],
                                lhsT,
                                xT(kt, ih * 512, 512),
                                start=(kt == 0),
                                stop=(kt == NKT - 1),
                            )
                    cp = nc.scalar.copy if evac_eng == "scalar" else nc.vector.tensor_copy
                    for lo, hi in split or [(0, N)]:
                        cp(dst[:, lo:hi], ps[:, lo:hi])

                def gproj(mt):
                    ps = psA.tile([128, N], fp32, tag="big", name="psg")
                    for kt in range(NKT):
                        lhsT = wg_(kt, mt)
                        for ih in range(2):
                            nc.tensor.matmul(
                                ps[:, ih * 512 : ih * 512 + 512],
                                lhsT,
                                xT(kt, ih * 512, 512),
                                start=(kt == 0),
                                stop=(kt == NKT - 1),
                            )
                    nc.scalar.activation(
                        gT_sb[mt][:], ps[:], Identity, bias=bg_sb[:, mt : mt + 1]
                    )

                # ---- v natural [token, d] + ones column per head (bf16) ----
                vhat_all = projpool.tile([128, NJT * HL * 65], bf16, tag="vhat")
                ones_view = vhat_all[:].rearrange(
                    "p (j h c) -> p j h c", j=NJT, c=65
                )[:, :, :, 64:65]
                nc.vector.memset(ones_view, 1.0)

                def vproj(jt):
                    vv = vhat_all[:, jt * HL * 65 : (jt + 1) * HL * 65].rearrange(
                        "p (h c) -> p h c", h=HL
                    )
                    ps2 = psA.tile([128, HDL], fp32, tag="big", name="ps2")
                    for kt in range(NKT):
                        nc.tensor.matmul(
                            ps2[:],
                            xT(kt, jt * 128, 128),
                            wv(kt),
                            start=(kt == 0),
                            stop=(kt == NKT - 1),
                        )
                    nc.vector.tensor_copy(
                        vv[:, :, 0:64], ps2[:].rearrange("p (h c) -> p h c", h=HL)
                    )

                # split evacs so the first QK (needs qT ih0 + kT cols 0:128)
                # unblocks as early as possible
                projqk(0, 0, "vector", split=[(0, 512), (512, N)])
                projqk(1, 0, "vector", split=[(0, 128), (128, N)])

                # ---- shared state across pairs ----
                U_sb = {}
                ug_sb = [
                    workpool.tile([128, N], bf16, tag=f"ug{p}", name=f"ug{p}", bufs=2)
                    for p in range(2)
                ]
                state = {}

                def attn_pair(p, background):
                    """jt-loop for head-pair p. AV matmuls run 3 (jt, hh)
                    units behind their QK so the in-order PE never waits on
                    the ACT-exp / DVE-mult chain; background thunks fill the
                    remaining PE slack (one slot per unit)."""
                    bgi = iter(background)
                    uv = [
                        psB.tile([65, N], fp32, tag="uv", name=f"uv{p}_{i}")
                        for i in range(2)
                    ]
                    pend = []

                    def flush_av():
                        jt0, hh0, e0 = pend.pop(0)
                        h = 2 * p + hh0
                        base = jt0 * HL * 65 + h * 65
                        for ih in range(2):
                            nc.tensor.matmul(
                                uv[hh0][:, ih * 512 : ih * 512 + 512],
                                vhat_all[:, base : base + 65],
                                e0[:, ih * 512 : ih * 512 + 512],
                                start=(jt0 == 0),
                                stop=(jt0 == NJT - 1),
                            )

                    for jt in range(NJT):
                        bt = bias_tiles[(p, jt)]
                        for hh in range(2):
                            st = psA.tile([128, N], fp32, tag="big", name=f"st{jt}_{hh}")
                            lhsT = kT_sb[p][hh * 64 : hh * 64 + 64, jt * 128 : jt * 128 + 128]
                            for ih in range(2):
                                nc.tensor.matmul(
                                    st[:, ih * 512 : ih * 512 + 512],
                                    lhsT,
                                    qT_sb[p][hh * 64 : hh * 64 + 64, ih * 512 : ih * 512 + 512],
                                    start=True,
                                    stop=True,
                                )
                            e1 = epool.tile([128, N], bf16, tag="e1", name="e1", bufs=3)
                            nc.scalar.activation(e1[:], st[:], Exp)
                            e = epool.tile([128, N], bf16, tag="e", name="e", bufs=6)
                            nc.vector.tensor_tensor(
                                out=e[:],
                                in0=e1[:],
                                in1=bt[:, hh * N : (hh + 1) * N],
                                op=mult,
                            )
                            pend.append((jt, hh, e))
                            if len(pend) > 4:
                                flush_av()
                            th = next(bgi, None)
                            if th is not None:
                                th()
                    for th in bgi:
                        if th is not None:
                            th()
                    return uv, pend, flush_av

                def epi_steps(p, hh, uv):
                    """Divide-by-denominator + gating for (p, hh). For (0,0)
                    the U*gT product runs on GPSIMD in parallel with the
                    reciprocal+broadcast chain (SBUF-SBUF TTs must share a
                    base partition, so only hh==0 qualifies). Elsewhere:
                    gs = broadcast(1/den)*gT (PSUM operand, exempt), then
                    ug = U*gs."""
                    par = (p, hh) == (0, 0)

                    def src():
                        return U_sb[(p, hh)] if p == 0 else uv[hh]

                    def s1():
                        rec = workpool.tile([1, N], f32r, tag="rec", name="rec", bufs=2)
                        with nc.allow_low_precision(reason="feeds PE broadcast"):
                            nc.vector.reciprocal(rec[:], src()[64:65, :])
                        state[("rec", p, hh)] = rec

                    def s1b():
                        ugp = workpool.tile([64, N], fp32, tag="gs", name="ugp", bufs=2)
                        nc.gpsimd.tensor_tensor(
                            out=ugp[:],
                            in0=src()[0:64, :],
                            in1=gT_sb[p][0:64, :],
                            op=mult,
                        )
                        state[("ugp", p, hh)] = ugp

                    def s2():
                        rec = state[("rec", p, hh)]
                        bc = psA.tile([64, N], fp32, tag="big", name="bc")
                        for ih in range(2):
                            nc.tensor.matmul(
                                bc[:, ih * 512 : ih * 512 + 512],
                                ones_sb[0:1, 0:64],
                                rec[0:1, ih * 512 : ih * 512 + 512],
                                start=True,
                                stop=True,
                            )
                        state[("bc", p, hh)] = bc
                        if not par:
                            gs = workpool.tile([64, N], fp32, tag="gs", name="gs", bufs=2)
                            nc.vector.tensor_tensor(
                                out=gs[:],
                                in0=bc[:],
                                in1=gT_sb[p][hh * 64 : hh * 64 + 64, :],
                                op=mult,
                            )
                            state[("gs", p, hh)] = gs

                    def s3():
                        if par:
                            nc.vector.tensor_tensor(
                                out=ug_sb[p][hh * 64 : hh * 64 + 64, :],
                                in0=state[("ugp", p, hh)][:],
                                in1=state[("bc", p, hh)][:],
                                op=mult,
                            )
                        else:
                            nc.vector.tensor_tensor(
                                out=ug_sb[p][hh * 64 : hh * 64 + 64, :],
                                in0=src()[0:64, :],
                                in1=state[("gs", p, hh)][:],
                                op=mult,
                            )

                    return ([s1, s1b, s2, s3] if par else [s1, s2, s3])

                def outproj_step(p, mt):
                    ps = psA.tile([128, N], fp32, tag="big", name="po")
                    lhsT = wo_(p, mt)
                    for ih in range(2):
                        nc.tensor.matmul(
                            ps[:, ih * 512 : ih * 512 + 512],
                            lhsT,
                            ug_sb[p][:, ih * 512 : ih * 512 + 512],
                            start=True,
                            stop=True,
                        )
                    ot = workpool.tile([128, N], bf16, tag="osb", name="osb", bufs=4)
                    if p == 1 or mt % 2 == 0:  # ACT is idle in the tail
                        nc.scalar.copy(ot[:], ps[:])
                    else:
                        nc.vector.tensor_copy(ot[:], ps[:])
                    nc.gpsimd.dma_start(outT[p, mt], ot[:])

                def outproj_steps(p):
                    return [
                        (lambda p=p, mt=mt: outproj_step(p, mt)) for mt in range(4)
                    ]

                # ---- pair 0: backgrounds = v/g projections + pair-1 q/k ----
                bg0 = (
                    [lambda j=j: vproj(j) for j in range(NJT)]
                    + [
                        lambda: projqk(0, 1, "vector"),
                        lambda: projqk(1, 1, "vector"),
                        lambda: gproj(0),
                    ]
                )
                uv0, pend, flush = attn_pair(0, bg0)
                while pend:
                    flush()
                for hh in range(2):
                    U = workpool.tile([65, N], fp32, tag="U", name=f"U0_{hh}", bufs=4)
                    if hh == 0:
                        nc.scalar.copy(U[:], uv0[hh][:])
                    else:
                        nc.vector.tensor_copy(U[:], uv0[hh][:])
                    U_sb[(0, hh)] = U

                # ---- pair 1: backgrounds = pair-0 epilogue/outproj ----
                e00 = epi_steps(0, 0, uv0)
                e01 = epi_steps(0, 1, uv0)
                op0 = outproj_steps(0)
                bg1 = [lambda: gproj(1)] + e00 + [None] + e01 + [None] + op0
                uv1, pend, flush = attn_pair(1, bg1)
                # Tail: drain, then both heads' epilogues with maximal
                # engine parallelism.
                while pend:
                    flush()
                e10 = epi_steps(1, 0, uv1)
                e11 = epi_steps(1, 1, uv1)
                e10[0]()  # recip hh0 (DVE)
                e11[0]()  # recip hh1 (DVE)

                # keep-warm matmuls: without them the PE idles through the
                # recip/gs/ug chain, drops its p-state, and the final
                # outproj runs at 1/2..1/4 clock. Write-only single tiles
                # (closed groups, no readers) keep the psA ring hazard-free.
                def warm(k):
                    pwt = psA.tile([1, 512], fp32, tag="big", name="pwt")
                    for _ in range(k):
                        nc.tensor.matmul(
                            pwt[:], wtl[0:1, 0:1], wtl[:], start=True, stop=True
                        )

                warm(8)
                e10[1]()  # broadcast hh0 (PE) + gs hh0 (DVE)
                e11[1]()  # broadcast hh1 (PE) + gs hh1 (DVE)
                e10[2]()  # ug hh0 (DVE)
                warm(6)
                e11[2]()  # ug hh1 (DVE)
                # outproj p1 on four independent PSUM tiles (two psA slots,
                # plus the two freed uv slots in psB) -> no ring-paced waits;
                # evacs split ACT+DVE; out-DMAs split across both rings.
                po_tiles = [
                    psA.tile([128, N], fp32, tag="big", name="po0"),
                    psA.tile([128, N], fp32, tag="big", name="po1"),
                    psB.tile([128, N], fp32, tag="uv", name="po2"),
                    psB.tile([128, N], fp32, tag="uv", name="po3"),
                ]
                for mt in range(4):
                    ps = po_tiles[mt]
                    lhsT = wo_(1, mt)
                    for ih in range(2):
                        nc.tensor.matmul(
                            ps[:, ih * 512 : ih * 512 + 512],
                            lhsT,
                            ug_sb[1][:, ih * 512 : ih * 512 + 512],
                            start=True,
                            stop=True,
                        )
                    ot = workpool.tile([128, N], bf16, tag="osb", name="osb", bufs=4)
                    nc.scalar.copy(ot[:, 0:512], ps[:, 0:512])
                    nc.vector.tensor_copy(ot[:, 512:N], ps[:, 512:N])
                    ring = nc.sync if mt % 2 == 0 else nc.gpsimd
                    ring.dma_start(outT[1, mt], ot[:])

            for i in range(unroll):
                body(first=(i == 0))

    nc.compile()
    return nc


def _shard_inputs(x, attn_bias, Wq, Wkv, Wg, bg, Wo):
    """Build per-core input maps (host-side layout prep)."""
    import ml_dtypes

    bf16 = ml_dtypes.bfloat16

    def kmaj(w):  # [512, F] -> [128, NKT*F] contraction-tile-major
        f = w.shape[1]
        return np.ascontiguousarray(
            w.reshape(NKT, 128, f).transpose(1, 0, 2)
        ).reshape(128, NKT * f)

    in_maps = []
    for d in range(NCORES):
        b, g = d // 2, d % 2
        cs = slice(g * HDL, (g + 1) * HDL)
        xTh = np.ascontiguousarray(x[b].T)  # [512, 1024]
        wq_all = Wq[:, cs] * SCALE  # [512, 256]
        wk_all = Wkv[:, g * HDL : (g + 1) * HDL]
        wv_all = Wkv[:, H * DH + g * HDL : H * DH + (g + 1) * HDL]
        wg_all = Wg[:, cs]
        chunks = [
            kmaj(wq_all[:, 0:128]),
            kmaj(wk_all[:, 0:128]),
            kmaj(xTh),
            kmaj(wv_all),
            kmaj(wq_all[:, 128:256]),
            kmaj(wk_all[:, 128:256]),
            kmaj(wg_all),
            np.ascontiguousarray(
                Wo[cs, :].reshape(2, 128, DIM).transpose(1, 0, 2).reshape(128, 2 * DIM)
            ),
        ]
        wpack = np.concatenate(chunks, axis=1).astype(bf16)
        wsmall = np.concatenate(
            [
                np.ascontiguousarray(bg[cs].reshape(2, 128).T),
                np.ones((128, 64), np.float32),
            ],
            axis=1,
        ).astype(np.float32)

        ab = attn_bias[b, g * HL : (g + 1) * HL]  # [4, N(i), N(j)]
        abT = ab.transpose(0, 2, 1).reshape(2, 2, NJT, 128, N)  # [p, hh, jt, jpart, i]
        eB = np.exp(abT.transpose(0, 2, 3, 1, 4))  # [p, jt, jpart, hh, i]
        expB = np.ascontiguousarray(eB).reshape(2, NJT, 128, 2 * N).astype(bf16)
        in_maps.append({"wpack": wpack, "wsmall": wsmall, "expB": expB})
    return in_maps


def _unshard(results, bo):
    out = np.empty((B, N, DIM), dtype=np.float32)
    for b in range(B):
        acc = results[2 * b]["outT"].astype(np.float32).sum(axis=0) + results[
            2 * b + 1
        ]["outT"].astype(np.float32).sum(axis=0)
        out[b] = acc.reshape(DIM, N).T + bo[None, :]
    return out


def kernel(x, mask, attn_bias, Wq, Wkv, Wg, bg, Wo, bo):
    """Full inputs in, full output out. mask is all-ones by construction."""
    from concourse.bass_utils import run_bass_kernel_spmd

    x = np.asarray(x, dtype=np.float32)
    attn_bias = np.asarray(attn_bias, dtype=np.float32)
    Wq = np.asarray(Wq, dtype=np.float32)
    Wkv = np.asarray(Wkv, dtype=np.float32)
    Wg = np.asarray(Wg, dtype=np.float32)
    bg = np.asarray(bg, dtype=np.float32)
    Wo = np.asarray(Wo, dtype=np.float32)
    bo = np.asarray(bo, dtype=np.float32)

    if "nc" not in _CACHE:
        _CACHE["nc"] = _build()
    in_maps = _shard_inputs(x, attn_bias, Wq, Wkv, Wg, bg, Wo)
    res = run_bass_kernel_spmd(_CACHE["nc"], in_maps, core_ids=list(range(NCORES)))
    return _unshard(res.results, bo)

